# revision 40
# baseline (speedup 1.0000x reference)
"""Multi-head causal attention on 8 Trainium2 NeuronCores.

Problem: resid_pre [4, 2048, 1024], 16 heads x d_head 64, causal softmax,
output [4, 2048, 1024] f32.

Sharding: data-parallel over the 4 batches x tensor-parallel over 2 head
groups (8 heads each) -> 8 cores. Each core computes the attention output
contribution of its 8 heads for its batch; the host sums the two head-group
partials per batch (the "all-reduce") and adds the output bias.

Per-core kernel (QK projections fp8 DoubleRow, other matmul inputs bf16,
all accumulation fp32 in PSUM; measured ~1.2e-2 max rel err vs the fp32
reference, threshold 2e-2):

  prelude, pipelined by 512-column blocks of X^T (causality means attention
  superblock sb only needs Q/K columns <= (sb+1)*512):
    V = X @ W_v for all 8 heads in natural [seq, d] layout with a ones
    column appended per head, and Q^T/K^T for head pair 0, pair-stacked on
    partitions (head 2p in partitions 0-63, 2p+1 in 64-127). All bulk
    tensors arrive as ONE large 2D DMA per block (a dma_start costs ~0.6us
    of issuing-engine queue time, so many small transfers serialize the
    prelude on the trigger queues, not on HBM bandwidth). A short stream of
    junk matmuls at the very start flips the PE HAM clock gate (1.2 GHz ->
    2.4 GHz) while the first DMAs land.

  per head pair p (heads 2p, 2p+1), per 512-wide query superblock, per
  128-wide key tile:
    S^T = K^T.T @ Q^T (keys on partitions, one matmul per head via
    partition row groups, which the PE runs concurrently), restricted to
    the un-masked column suffix; exp on ScalarE (no max subtraction
    needed, scores are O(1)); the diagonal block's upper triangle is then
    zeroed by one DVE bf16 multiply with a 0/1 mask (cheaper than PE
    identity-matmul mask adds, whose streams the PE serializes);
    z~^T[65, 512] += V_chunk.T @ P~^T accumulated in PSUM, whose row 64
    (from the ones column) is the softmax denominator; normalize with
    reciprocal_approx_fast + gpsimd partition broadcast. Pair p+1's Q/K
    projection matmuls (fp8 DoubleRow: two 128-row d_model chunks per
    instruction at 0.5 cyc/col) are interleaved BETWEEN each step's score
    and PV matmuls (the PE queue is strict FIFO, so fillers there cover
    the exp/mask latency), holding a few back past the superblock-3 tail
    so the pair-boundary normalize also has PE work (else the PE idles
    >3.4us and the HAM clock gate re-throttles the next pair); for the
    last pair the output projection tiles of already-final superblocks are
    interleaved instead, with 3 held back to cover the final normalize,
    which is chunked by 128 columns and pipelined with the last projection
    waves.

  output projection: out[q, m] = sum_p z^T_p.T @ W_o_p into the shared
  projection-psum tag, PSUM -> SBUF (bf16) -> DRAM. A single pool scope
  covers the whole kernel: no mid-kernel pool-close DRAIN barriers.

b_Q/b_K are applied on-device (per-partition bias during the PSUM->SBUF
copy); b_V's exact contribution sum_h W_O[h].T @ b_V[h] (softmax rows sum
to 1) and b_O are added on the host, which also upcasts the bf16 partial
outputs and sums the two head groups.
"""
import ml_dtypes
import numpy as np

import concourse.bass as bass
import concourse.mybir as mybir
import concourse.tile as tile
from concourse import bacc
from concourse import bass_utils

F32 = mybir.dt.float32
F8 = mybir.dt.float8e4
DROW = mybir.MatmulPerfMode.DoubleRow
EXPF = mybir.ActivationFunctionType.Exp

S = 2048          # sequence length
DM = 1024         # d_model
DH = 64           # d_head
NHC = 8           # heads per core
PAIRS = 4         # head pairs per core
MC = 8            # d_model chunks of 128
MC2 = 4           # d_model chunk PAIRS of 256 (fp8 DoubleRow)
NSB = 4           # query superblocks of 512
SBW = 512         # superblock width
NKT = 16          # key tiles of 128
NST = 16          # seq tiles of 128
MASK_NEG = -1e9
WSCALE = 8.0      # fp8 Q/K weight pre-scale (lifts W out of e4m3 subnormals)
SCALE = 0.125 / (WSCALE * WSCALE)   # 1/sqrt(d_head), undoing the W pre-scale
N_WARM = 9        # junk N=512 warm-up matmuls to flip the PE HAM gate early

_NC_CACHE = {}
LAST_RESULTS = None


def _build_nc():
    nc = bacc.Bacc("TRN2", target_bir_lowering=False, debug=False)
    BF16 = mybir.dt.bfloat16
    xt_d = nc.dram_tensor("xt", [NSB, 128, MC, SBW], BF16, kind="ExternalInput")
    xt8_d = nc.dram_tensor("xt8", [NSB, 128, MC2, 2, SBW], F8, kind="ExternalInput")
    wq_d = nc.dram_tensor("wq", [PAIRS, 128, MC2, 2, 128], F8, kind="ExternalInput")
    wk_d = nc.dram_tensor("wk", [PAIRS, 128, MC2, 2, 128], F8, kind="ExternalInput")
    wv_d = nc.dram_tensor("wv", [128, MC, 512], BF16, kind="ExternalInput")
    wo_d = nc.dram_tensor("wo", [128, PAIRS, DM], BF16, kind="ExternalInput")
    bq_d = nc.dram_tensor("bq", [PAIRS, 128, 1], F32, kind="ExternalInput")
    bk_d = nc.dram_tensor("bk", [PAIRS, 128, 1], F32, kind="ExternalInput")
    msk_d = nc.dram_tensor("mask", [128, 256], BF16, kind="ExternalInput")
    out_d = nc.dram_tensor("out", [S, DM], BF16, kind="ExternalOutput")

    with tile.TileContext(nc) as tc:
      with (
          tc.tile_pool(name="hold", bufs=1) as hold,
          tc.tile_pool(name="ph2", bufs=1) as ph2,
          tc.tile_pool(name="patn", bufs=1, space="PSUM") as patn,
          tc.tile_pool(name="ph1", bufs=1) as ph1,
          tc.tile_pool(name="pqk", bufs=1, space="PSUM") as pqk,
      ):
        v_t = [hold.tile([128, NHC, DH + 1], BF16, tag=f"v{st}", name=f"v{st}") for st in range(NST)]
        z_t = [hold.tile([128, S], BF16, tag=f"z{p}", name=f"z{p}") for p in range(PAIRS)]
        msk_t = hold.tile([128, 256], BF16, tag="mtri")
        bq_t = [hold.tile([128, 1], F32, tag=f"bq{p}", name=f"bq{p}") for p in range(PAIRS)]
        bk_t = [hold.tile([128, 1], F32, tag=f"bk{p}", name=f"bk{p}") for p in range(PAIRS)]
        ones_c = hold.tile([128, 1], F32, tag="ones")
        qts = {}

        nc.vector.memset(ones_c[:], 1.0)
        # small constants go through the gpsimd DMA queue (only wo shares it,
        # and wo isn't needed until the last-pair phase) so their ~0.7us
        # triggers don't delay the xt/wv bulk loads on the sync/scalar queues
        nc.gpsimd.dma_start(msk_t[:], msk_d.ap())
        for p in range(PAIRS):
            nc.gpsimd.dma_start(bq_t[p][:], bq_d.ap()[p])
            nc.gpsimd.dma_start(bk_t[p][:], bk_d.ap()[p])

        # The PE HAM clock gate keeps the array at 1.2 GHz until it has been
        # continuously busy for a full ~3.4us activity window. The first few
        # us are framework preamble + DMA waits anyway, so run a continuous
        # stream of junk matmuls on a zeroed tile to flip the gate before the
        # real matmuls start (each N=512 matmul is ~426ns at the cold clock).
        warm_t = hold.tile([128, 512], BF16, tag="warm")
        nc.vector.memset(warm_t[:], 0.0)
        for _ in range(N_WARM):
            wp = patn.tile([128, 1024], F32, tag="sp", bufs=2, name="warm")
            nc.tensor.matmul(wp[0:16, 0:512], warm_t[:, 0:16], warm_t[:],
                             start=True, stop=True)

        def attn_scores(p, sb, j):
            """Scores + mask + exp for step j; PV is issued separately so
            filler matmuls can sit between them in the PE's strict FIFO,
            covering the exp latency."""
            qt, kt = qts[p]
            qtb = qt[sb]
            ktb = kt[j // 4]
            # columns q < j*128 of this key tile are fully masked;
            # restrict S/exp/PV to the valid suffix.
            j_rel = j - 4 * sb
            off = max(j_rel, 0) * 128
            sp = patn.tile([128, 1024], F32, tag="sp", bufs=2, name="sp")
            ks = ((j % 4) * 128, (j % 4 + 1) * 128)
            diag = j_rel >= 0
            nc.tensor.matmul(
                sp[:, off:512],
                ktb[0:64, ks[0]:ks[1]],
                qtb[0:64, off:SBW],
                start=True, stop=True,
                tile_position=(0, 0),
                skip_group_check=True,
            )
            nc.tensor.matmul(
                sp[:, 512 + off:1024],
                ktb[64:128, ks[0]:ks[1]],
                qtb[64:128, off:SBW],
                start=True, stop=True,
                tile_position=(64, 0),
                skip_group_check=True,
            )
            pt = ph2.tile([128, 1024], BF16, tag="pt", bufs=6, name="pt")
            if off == 0:
                # contiguous suffix: flat 2D AP is slightly cheaper on ACT
                nc.scalar.activation(pt[:, 0:1024], sp[:, 0:1024], EXPF, scale=SCALE)
            else:
                sp3 = sp[:].rearrange("p (u q) -> p u q", u=2)
                pt3 = pt[:].rearrange("p (u q) -> p u q", u=2)
                nc.scalar.activation(
                    pt3[:, :, off:512], sp3[:, :, off:512], EXPF, scale=SCALE
                )
            if diag:
                # zero the masked upper triangle of the diagonal prob block
                # for both heads with one DVE bf16 multiply (off the PE)
                pt3m = pt[:].rearrange("p (u q) -> p u q", u=2)
                tri3 = msk_t[:].rearrange("p (u q) -> p u q", u=2)
                nc.vector.tensor_mul(
                    pt3m[:, :, off:off + 128], pt3m[:, :, off:off + 128], tri3[:]
                )
            return pt, off

        def attn_pv(p, sb, j, z0, z1, pt, off):
            nkt = 4 * (sb + 1)
            nc.tensor.matmul(
                z0[:, off:512],
                v_t[j][:, 2 * p, :],
                pt[:, off:512],
                start=(j == 0), stop=(j == nkt - 1),
            )
            nc.tensor.matmul(
                z1[:, off:512],
                v_t[j][:, 2 * p + 1, :],
                pt[:, 512 + off:1024],
                start=(j == 0), stop=(j == nkt - 1),
            )

        def attn_j(p, sb, j, z0, z1, fill=None):
            pt, off = attn_scores(p, sb, j)
            if fill is not None:
                fill()
            attn_pv(p, sb, j, z0, z1, pt, off)

        pending_norm = []

        def norm_finish(p, sb, s0, s1, c0=0, c1=512, direct=False):
            # the reciprocal/broadcast/multiply part of the normalize. For
            # non-final superblocks this is emitted a few steps into the NEXT
            # superblock (norm_pop): its DVE ops otherwise sit in the DVE
            # FIFO ahead of the diagonal-step mask multiplies (pair
            # boundaries land on superblock 0, which is ALL diagonal steps)
            # and stall their PVs by ~2us.
            w = c1 - c0
            qs = (sb * SBW + c0, sb * SBW + c1)
            d0row = ph2.tile([1, 512], F32, tag="d0row", bufs=2, name="d0row")
            d1row = ph2.tile([1, 512], F32, tag="d1row", bufs=2, name="d1row")
            cc0, cc1 = (c0, c1) if direct else (0, w)
            nc.vector.tensor_copy(d0row[:, 0:w], s0[DH:DH + 1, cc0:cc1])
            nc.vector.tensor_copy(d1row[:, 0:w], s1[DH:DH + 1, cc0:cc1])
            nc.vector.reciprocal_approx_fast(d0row[:, 0:w], d0row[:, 0:w])
            nc.vector.reciprocal_approx_fast(d1row[:, 0:w], d1row[:, 0:w])
            r0 = ph2.tile([64, 512], F32, tag="r0", bufs=2, name="r0")
            r1 = ph2.tile([64, 512], F32, tag="r1", bufs=2, name="r1")
            nc.gpsimd.partition_broadcast(r0[:, 0:w], d0row[:, 0:w], channels=64)
            nc.gpsimd.partition_broadcast(r1[:, 0:w], d1row[:, 0:w], channels=64)
            # deferred finishes do the big multiplies on GPSIMD: on DVE they
            # would sit ahead of the next superblock's diagonal-step mask
            # multiplies in the strict FIFO and stall their PVs ~2us. The
            # latency-critical endgame chunks (direct=True) stay on DVE.
            mul = nc.vector.tensor_mul if direct else nc.gpsimd.tensor_mul
            mul(z_t[p][0:64, qs[0]:qs[1]], s0[0:64, cc0:cc1], r0[:, 0:w])
            t1 = ph2.tile([64, 512], BF16, tag="t1", bufs=2, name="t1")
            mul(t1[:, 0:w], s1[0:64, cc0:cc1], r1[:, 0:w])
            nc.sync.dma_start(z_t[p][64:128, qs[0]:qs[1]], t1[:, 0:w])

        def attn_norm(p, sb, z0, z1, q0=0, q1=512, direct=False, defer=False):
            # normalize by the softmax denominator (row DH of z psum).
            # First copy z psum to SBUF so the bank frees immediately (the
            # next superblock's PV only waits for this copy, not the whole
            # reciprocal/broadcast/multiply chain). Optional [q0,q1) restricts
            # to a column chunk so the tail can pipeline norm with oproj.
            if direct:
                # tail chunks: nothing runs after, so bank release doesn't
                # matter -- read the z psum directly (shortest chain)
                norm_finish(p, sb, z0, z1, q0, q1, direct=True)
                return
            # copy the whole z psum (a [65,w] partition-parallel copy
            # costs ~the same as a [1,w] one) so each bank frees after
            # one DVE op instead of holding through the whole chain
            zc0 = ph2.tile([DH + 1, 512], F32, tag="zc0", bufs=2, name="zc0")
            zc1 = ph2.tile([DH + 1, 512], F32, tag="zc1", bufs=2, name="zc1")
            nc.vector.tensor_copy(zc0[:, 0:512], z0[:, 0:512])
            nc.vector.tensor_copy(zc1[:, 0:512], z1[:, 0:512])
            if defer:
                pending_norm.append((p, sb, zc0, zc1))
            else:
                norm_finish(p, sb, zc0, zc1)

        def norm_pop():
            # one pending per site: emitting two full-width finish chains at
            # once overruns the short superblock-1 window
            if pending_norm:
                norm_finish(*pending_norm.pop(0))

        # xt in per-512-column-block tiles: attention(0, sb) needs only
        # Q/K columns <= (sb+1)*512 (causal), so the whole front of the
        # kernel pipelines by column block. Each block arrives as two big
        # DMAs (one per trigger queue); fp8 copies feed the QK DoubleRow
        # projections. fp8 column block 3 lives in the long-lived pool:
        # pair-3's sb-3 projection is deferred into the last-pair phase.
        xt_t = [[ph1.tile([128, MC // 2, SBW], BF16, tag=f"xt{h}_{cb}",
                          name=f"xt{h}_{cb}")
                 for h in range(2)] for cb in range(NSB)]
        xt8_t = [(hold if cb == 3 else ph1).tile(
                     [128, MC2, 2, SBW], F8, tag=f"x8{cb}", name=f"x8{cb}")
                 for cb in range(NSB)]
        wv_t = [ph1.tile([128, MC // 2, 512], BF16, tag=f"wv{h}", name=f"wv{h}")
                for h in range(2)]
        wo_t = ph1.tile([128, PAIRS, DM], BF16, tag="wo", name="wo")

        def qk_gen(p, sb_outer=False, defer_sb3=False):
            """QK projection for pair p (fp8 DoubleRow, pair-stacked
            partitions), yielded one matmul at a time for interleaving.
            Weight DMAs issue eagerly at creation; the matmuls come from the
            returned generator. With sb_outer=True the superblock loop is
            outermost so early superblocks finish as soon as their xt column
            block lands. defer_sb3 skips superblock 3 (emitted later, see
            last pair)."""
            qt = [hold.tile([128, SBW], mybir.dt.bfloat16, tag=f"qt{i}", bufs=2,
                            name=f"qt{i}")
                  for i in range(NSB)]
            kt = [hold.tile([128, SBW], mybir.dt.bfloat16, tag=f"kt{i}", bufs=2,
                            name=f"kt{i}")
                  for i in range(NSB)]
            qts[p] = (qt, kt)
            wqk = []
            for qi, (w_d, b_t, dst) in enumerate(
                    ((wq_d, bq_t, qt), (wk_d, bk_t, kt))):
                w = ph1.tile([128, MC2, 2, 128], F8, tag="w", bufs=4, name="w")
                (nc.sync if qi == 0 else nc.scalar).dma_start(w[:], w_d.ap()[p])
                wqk.append((w, b_t, dst))
            order = (
                [(sb, wb) for sb in range(NSB) for wb in wqk]
                if sb_outer else
                [(sb, wb) for wb in wqk for sb in range(NSB)]
            )
            if defer_sb3:
                order = [o for o in order if o[0] != 3]

            def gen():
                for sb, (w, b_t, dst) in order:
                    ps = pqk.tile([128, 512], F32, tag="acc", bufs=2, name="acc")
                    for mp in range(MC2):
                        nc.tensor.matmul(
                            ps[:],
                            w[:, mp, :, :],
                            xt8_t[sb][:, mp, :, :],
                            start=(mp == 0),
                            stop=(mp == MC2 - 1),
                            perf_mode=DROW,
                        )
                        yield
                    nc.vector.tensor_scalar_add(dst[sb][:], ps[:], b_t[p][:])
                    yield

            return gen()

        # column-block pipelined prelude: per block, land xt columns (two
        # half-block DMAs on separate trigger queues + one fp8 DMA), then
        # V-projection for its 4 seq tiles, pair 0's QK for it, and pair 0's
        # attention superblock cb. Interleaving pair-0 attention into the
        # prelude hides the next block's DMA latency (per-queue DMA delivery
        # is only ~0.25 MB/us) behind the ACT-paced softmax steps.
        # wv halves land first (the first V matmul needs them), wo much later
        nc.sync.dma_start(wv_t[0][:], wv_d.ap()[:, 0:MC // 2, :])
        nc.scalar.dma_start(wv_t[1][:], wv_d.ap()[:, MC // 2:MC, :])
        nc.gpsimd.dma_start(wo_t[:], wo_d.ap())
        g0 = qk_gen(0, sb_outer=True)
        state1 = {"done": False, "emitted": 0, "step": 0, "g": None, "cb": 0}

        def fill1(state=state1):
            # pair-1 projection filler inside pair-0's attention: capped by
            # the xt8 blocks that have landed (superblock-outer order, 10
            # items per block), front-loaded 2/step with 8 held back and 4
            # of those released over the last steps
            state["step"] += 1
            step = state["step"]
            want = min(2 * step, 10 * (state["cb"] + 1), 32) + max(0, step - 36)
            while state["emitted"] < want and not state["done"]:
                try:
                    next(state["g"])
                    state["emitted"] += 1
                except StopIteration:
                    state["done"] = True

        def issue_cb_dmas(cb):
            nc.sync.dma_start(xt_t[cb][0][:], xt_d.ap()[cb][:, 0:MC // 2, :])
            nc.scalar.dma_start(xt_t[cb][1][:], xt_d.ap()[cb][:, MC // 2:MC, :])
            (nc.scalar if cb % 2 == 0 else nc.sync).dma_start(
                xt8_t[cb][:], xt8_d.ap()[cb])

        issue_cb_dmas(0)
        for cb in range(NSB):
            if cb + 1 < NSB:
                # keep the next block's transfers in flight behind this
                # block's compute + attention
                issue_cb_dmas(cb + 1)
            for st in range(4 * cb, 4 * cb + 4):
                ps = pqk.tile([128, 512], F32, tag="acc", bufs=2, name="acc")
                for m in range(MC):
                    nc.tensor.matmul(
                        ps[:],
                        xt_t[cb][m // 4][:, m % 4,
                                         (st % 4) * 128:(st % 4 + 1) * 128],
                        wv_t[m // 4][:, m % 4, :],
                        start=(m == 0),
                        stop=(m == MC - 1),
                    )
                nc.vector.tensor_copy(
                    v_t[st][:, :, 0:DH],
                    ps[:].rearrange("p (h d) -> p h d", h=NHC),
                )
                nc.vector.tensor_copy(
                    v_t[st][:, :, DH],
                    ones_c[:].to_broadcast((128, NHC)),
                )
            for _ in range(10):  # one QK column-block (2 proj x (4 mm + copy))
                try:
                    next(g0)
                except StopIteration:
                    break
            if cb == 0:
                # pair 1's projection generator: weight DMAs issue now (they
                # queue behind block 0's bulk loads)
                state1["g"] = qk_gen(1, sb_outer=True)
            state1["cb"] = cb
            # pair 0's attention for superblock cb
            nkt = 4 * (cb + 1)
            z0 = patn.tile([DH + 1, 512], F32, tag="z0", bufs=1, name="z0")
            z1 = patn.tile([DH + 1, 512], F32, tag="z1", bufs=1, name="z1")
            for j in range(nkt):
                attn_j(0, cb, j, z0, z1, fill=fill1)
                if cb > 0 and j == 0:
                    norm_pop()
                if cb == 3 and j == 6:
                    norm_pop()
            attn_norm(0, cb, z0, z1, defer=True)
        for _ in g0:
            pass
        while not state1["done"]:
            try:
                next(state1["g"])
            except StopIteration:
                state1["done"] = True

        # attention for pairs 1-2, with pair p+1's projection matmuls
        # interleaved into the ACT-paced attention stream
        for p in (1, 2):
            g = qk_gen(p + 1, defer_sb3=(p == 2))
            total = 30 if p == 2 else 40
            state = {"done": False, "emitted": 0, "step": 0}

            def fill(state=state, g=g, total=total):
                # front-load 2/step, but hold 8 items back, releasing only 4
                # over the last steps: the remaining ~4 emit in the post-loop
                # drain, covering the pair-boundary normalize window
                state["step"] += 1
                step = state["step"]
                want = min(2 * step, total - 8) + max(0, step - 36)
                while state["emitted"] < want and not state["done"]:
                    try:
                        next(g)
                        state["emitted"] += 1
                    except StopIteration:
                        state["done"] = True

            for sb in range(NSB):
                nkt = 4 * (sb + 1)
                z0 = patn.tile([DH + 1, 512], F32, tag="z0", bufs=1, name="z0")
                z1 = patn.tile([DH + 1, 512], F32, tag="z1", bufs=1, name="z1")
                for j in range(nkt):
                    attn_j(p, sb, j, z0, z1, fill=fill)
                    if sb > 0 and j == 0:
                        norm_pop()
                    if sb == 3 and j == 6:
                        norm_pop()
                attn_norm(p, sb, z0, z1, defer=True)
            while not state["done"]:
                try:
                    next(g)
                except StopIteration:
                    state["done"] = True

        # ---------------- last pair + output projection ----------------
        # pair-3's sb-3 Q/K projection was deferred out of pair 2's phase;
        # emit it here as filler for pair-3's otherwise-empty early steps.
        wq3 = ph1.tile([128, MC2, 2, 128], F8, tag="wq3", name="wq3")
        wk3 = ph1.tile([128, MC2, 2, 128], F8, tag="wk3", name="wk3")
        nc.scalar.dma_start(wq3[:], wq_d.ap()[3])
        nc.sync.dma_start(wk3[:], wk_d.ap()[3])

        def deferred_proj():
            for wts, b_t, dst in ((wq3, bq_t, qts[3][0]), (wk3, bk_t, qts[3][1])):
                ps = pqk.tile([128, 512], F32, tag="acc", bufs=2, name="dps")
                for mp in range(MC2):
                    nc.tensor.matmul(
                        ps[:],
                        wts[:, mp, :, :],
                        xt8_t[3][:, mp, :, :],
                        start=(mp == 0),
                        stop=(mp == MC2 - 1),
                        perf_mode=DROW,
                    )
                    yield
                nc.vector.tensor_scalar_add(dst[3][:], ps[:], b_t[3][:])
                yield

        gdef = deferred_proj()
        gstate = {"done": False}

        def oproj(q, mb, cp=None):
            # shares the projection psum tag, so units never contend with
            # pair-3's score psum
            ps = pqk.tile([128, 512], F32, tag="acc", bufs=2, name="ops")
            for p in range(PAIRS):
                nc.tensor.matmul(
                    ps[:],
                    z_t[p][:, q * 128:(q + 1) * 128],
                    wo_t[:, p, mb * 512:(mb + 1) * 512],
                    start=(p == 0),
                    stop=(p == PAIRS - 1),
                )
            ost = ph1.tile([128, 512], mybir.dt.bfloat16, tag="ost", bufs=4,
                           name="ost")
            (cp or nc.vector.tensor_copy)(ost[:], ps[:])
            nc.sync.dma_start(
                out_d.ap()[q * 128:(q + 1) * 128, mb * 512:(mb + 1) * 512],
                ost[:],
            )

        # pair 3's attention, with output-projection tiles for already-
        # complete superblocks interleaved in (sb lags by one).
        otodo = [(q, mb) for q in range(NST) for mb in range(2)]
        odone = 0
        for sb in range(NSB):
            nkt = 4 * (sb + 1)
            z0 = patn.tile([DH + 1, 512], F32, tag="z0", bufs=1, name="z0")
            z1 = patn.tile([DH + 1, 512], F32, tag="z1", bufs=1, name="z1")

            stepc = {"j": 0}

            def fill(sb=sb, nkt=nkt):
                # first finish the deferred sb-3 projection (2 matmuls per
                # step, exclusive so it doesn't contend with units for the
                # acc psum); then z for superblocks < sb is final for all
                # pairs -- spread this superblock's unit quota evenly over
                # its steps (instead of draining immediately and leaving
                # the later steps with no filler), holding 2 back so the
                # final norm chain has PE work to hide behind
                nonlocal odone
                stepc["j"] += 1
                pumped = 0
                for _ in range(2):
                    if not gstate["done"]:
                        try:
                            next(gdef)
                            pumped += 1
                        except StopIteration:
                            gstate["done"] = True
                if pumped == 0:
                    ready = min(sb * 8, 22)
                    base = 8 * (sb - 1) if sb > 0 else 0
                    avail = ready - base
                    allowed = base + min(avail, (stepc["j"] * avail + nkt - 1) // nkt)
                    if odone < min(ready, allowed):
                        oproj(*otodo[odone])
                        odone += 1

            for j in range(nkt):
                attn_j(3, sb, j, z0, z1, fill=fill)
                if sb > 0 and j == 0:
                    norm_pop()
            if sb < NSB - 1:
                # pair-3 norms stay inline (not deferred): the oproj
                # interleave for superblock sb's q-tiles needs them promptly
                attn_norm(3, sb, z0, z1)
            else:
                # last superblock: chunk the normalize by 128 columns and
                # emit each q-tile's 2 output units right after its chunk,
                # so the norm chain pipelines with the final projections
                for c in range(4):
                    attn_norm(3, sb, z0, z1, q0=c * 128, q1=(c + 1) * 128, direct=True)
                    if c == 0:
                        # drain any interleave backlog (needs only sb<3 z,
                        # which is final) while chunk 0's norm chain runs;
                        # all endgame copies go to ScalarE (exp stream is
                        # done, and DVE must stay clear for the norm chain)
                        while odone < 24:
                            oproj(*otodo[odone], cp=nc.scalar.copy)
                            odone += 1
                    for i in range(2):
                        oproj(*otodo[odone],
                              cp=(nc.scalar.copy if i == 0 else None))
                        odone += 1

    nc.compile()
    return nc


def _get_nc():
    if "nc" not in _NC_CACHE:
        _NC_CACHE["nc"] = _build_nc()
    return _NC_CACHE["nc"]


def _causal_masks():
    # 0/1 keep-mask for the post-exp DVE multiply, one copy per head half
    k = np.arange(128)[:, None]
    q = np.arange(128)[None, :]
    tri = np.where(q >= k, 1.0, 0.0).astype(ml_dtypes.bfloat16)
    return np.concatenate([tri, tri], axis=1)


def kernel(resid_pre, W_Q, W_K, W_V, W_O, b_Q, b_K, b_V, b_O):
    global LAST_RESULTS
    resid_pre = np.asarray(resid_pre, dtype=np.float32)
    W_Q = np.asarray(W_Q, dtype=np.float32)
    W_K = np.asarray(W_K, dtype=np.float32)
    W_V = np.asarray(W_V, dtype=np.float32)
    W_O = np.asarray(W_O, dtype=np.float32)
    b_Q = np.asarray(b_Q, dtype=np.float32)
    b_K = np.asarray(b_K, dtype=np.float32)
    b_V = np.asarray(b_V, dtype=np.float32)
    b_O = np.asarray(b_O, dtype=np.float32)

    B = resid_pre.shape[0]
    masks = _causal_masks()
    BF = ml_dtypes.bfloat16
    F8NP = ml_dtypes.float8_e4m3

    def pack_xt(xt):  # [1024, 2048] -> [4, 128, 8, 512] bf16
        r = xt.reshape(MC, 128, NSB, SBW).transpose(2, 1, 0, 3)
        return np.ascontiguousarray(r).astype(BF)

    def pack_xt8(xt):  # [1024, 2048] -> [4, 128, 4, 2, 512] fp8
        r = xt.reshape(MC2, 2, 128, NSB, SBW).transpose(3, 2, 0, 1, 4)
        return np.ascontiguousarray(r).astype(F8NP)

    def pack_w8(w):  # [8, 1024, 64] -> [4, 128, 4, 2, 128] fp8, pre-scaled
        p8 = (w * WSCALE).reshape(PAIRS, 2, DM, DH).transpose(0, 2, 1, 3)
        p8 = p8.reshape(PAIRS, MC2, 2, 128, 128).transpose(0, 3, 1, 2, 4)
        return np.ascontiguousarray(p8).astype(F8NP)

    def pack_wv(w):  # [8, 1024, 64] -> [128, 8, 512] bf16
        r = w.transpose(1, 0, 2).reshape(DM, NHC * DH)
        r = r.reshape(MC, 128, NHC * DH).transpose(1, 0, 2)
        return np.ascontiguousarray(r).astype(BF)

    def pack_wo(w):  # [8, 64, 1024] -> [128, 4, 1024] bf16
        r = w.reshape(PAIRS, 128, DM).transpose(1, 0, 2)
        return np.ascontiguousarray(r).astype(BF)

    in_maps = []
    for c in range(8):
        b, g = divmod(c, 2)
        hs = slice(g * NHC, (g + 1) * NHC)
        xt = resid_pre[b].T
        in_maps.append({
            "xt": pack_xt(xt),
            "xt8": pack_xt8(xt),
            "wq": pack_w8(W_Q[hs]),
            "wk": pack_w8(W_K[hs]),
            "wv": pack_wv(W_V[hs]),
            "wo": pack_wo(W_O[hs]),
            "bq": np.ascontiguousarray(b_Q[hs].reshape(PAIRS, 128, 1)) * WSCALE,
            "bk": np.ascontiguousarray(b_K[hs].reshape(PAIRS, 128, 1)) * WSCALE,
            "mask": masks,
        })

    nc = _get_nc()
    res = bass_utils.run_bass_kernel_spmd(nc, in_maps, core_ids=list(range(8)))
    LAST_RESULTS = res

    # b_V contributes exactly sum_h W_O[h].T @ b_V[h] (softmax rows sum to 1)
    const = np.einsum("hdm,hd->m", W_O, b_V).astype(np.float32) + b_O
    out = np.empty((B, S, DM), dtype=np.float32)
    for b in range(B):
        out[b] = (res.results[2 * b]["out"].astype(np.float32)
                  + res.results[2 * b + 1]["out"].astype(np.float32) + const)
    return out


# revision 45
# speedup vs baseline: 1.9852x; 1.9852x over previous
"""Multi-head causal attention on 8 Trainium2 NeuronCores.

Problem: resid_pre [4, 2048, 1024], 16 heads x d_head 64, causal softmax,
output [4, 2048, 1024] f32.

Sharding: data-parallel over the 4 batches x tensor-parallel over 2 head
groups (8 heads each) -> 8 cores. Each core computes the attention output
contribution of its 8 heads for its batch; the host sums the two head-group
partials per batch (the "all-reduce") and adds the output bias.

Per-core kernel (QK projections fp8 DoubleRow, other matmul inputs bf16,
all accumulation fp32 in PSUM; measured ~1.2e-2 max rel err vs the fp32
reference, threshold 2e-2):

  prelude, pipelined by 512-column blocks of X^T (causality means attention
  superblock sb only needs Q/K columns <= (sb+1)*512):
    V = X @ W_v for all 8 heads in natural [seq, d] layout with a ones
    column appended per head, and Q^T/K^T for head pair 0, pair-stacked on
    partitions (head 2p in partitions 0-63, 2p+1 in 64-127). All bulk
    tensors arrive as ONE large 2D DMA per block (a dma_start costs ~0.6us
    of issuing-engine queue time, so many small transfers serialize the
    prelude on the trigger queues, not on HBM bandwidth). A short stream of
    junk matmuls at the very start flips the PE HAM clock gate (1.2 GHz ->
    2.4 GHz) while the first DMAs land.

  per head pair p (heads 2p, 2p+1), per 512-wide query superblock, per
  128-wide key tile:
    S^T = K^T.T @ Q^T (keys on partitions, one matmul per head via
    partition row groups, which the PE runs concurrently), restricted to
    the un-masked column suffix; exp on ScalarE (no max subtraction
    needed, scores are O(1)); the diagonal block's upper triangle is then
    zeroed by one DVE bf16 multiply with a 0/1 mask (cheaper than PE
    identity-matmul mask adds, whose streams the PE serializes);
    z~^T[65, 512] += V_chunk.T @ P~^T accumulated in PSUM, whose row 64
    (from the ones column) is the softmax denominator; normalize with
    reciprocal_approx_fast + gpsimd partition broadcast. Pair p+1's Q/K
    projection matmuls (fp8 DoubleRow: two 128-row d_model chunks per
    instruction at 0.5 cyc/col) are interleaved BETWEEN each step's score
    and PV matmuls (the PE queue is strict FIFO, so fillers there cover
    the exp/mask latency), holding a few back past the superblock-3 tail
    so the pair-boundary normalize also has PE work (else the PE idles
    >3.4us and the HAM clock gate re-throttles the next pair); for the
    last pair the output projection tiles of already-final superblocks are
    interleaved instead, with 3 held back to cover the final normalize,
    which is chunked by 128 columns and pipelined with the last projection
    waves.

  output projection: out[q, m] = sum_p z^T_p.T @ W_o_p into the shared
  projection-psum tag, PSUM -> SBUF (bf16) -> DRAM. A single pool scope
  covers the whole kernel: no mid-kernel pool-close DRAIN barriers.

b_Q/b_K are applied on-device (per-partition bias during the PSUM->SBUF
copy); b_V's exact contribution sum_h W_O[h].T @ b_V[h] (softmax rows sum
to 1) and b_O are added on the host, which also upcasts the bf16 partial
outputs and sums the two head groups.
"""
import ml_dtypes
import numpy as np

import concourse.bass as bass
import concourse.mybir as mybir
import concourse.tile as tile
from concourse import bacc
from concourse import bass_utils

F32 = mybir.dt.float32
F8 = mybir.dt.float8e4
DROW = mybir.MatmulPerfMode.DoubleRow
EXPF = mybir.ActivationFunctionType.Exp

S = 2048          # sequence length
DM = 1024         # d_model
DH = 64           # d_head
NHC = 8           # heads per core
PAIRS = 4         # head pairs per core
MC = 8            # d_model chunks of 128
MC2 = 4           # d_model chunk PAIRS of 256 (fp8 DoubleRow)
NSB = 4           # query superblocks of 512
SBW = 512         # superblock width
NKT = 16          # key tiles of 128
NST = 16          # seq tiles of 128
MASK_NEG = -1e9
WSCALE = 8.0      # fp8 Q/K weight pre-scale (lifts W out of e4m3 subnormals)
SCALE = 0.125 / (WSCALE * WSCALE)   # 1/sqrt(d_head), undoing the W pre-scale
N_WARM = 9        # junk N=512 warm-up matmuls to flip the PE HAM gate early

_NC_CACHE = {}
LAST_RESULTS = None


def _build_nc():
    nc = bacc.Bacc("TRN2", target_bir_lowering=False, debug=False)
    BF16 = mybir.dt.bfloat16
    xt_d = nc.dram_tensor("xt", [NSB, 128, MC, SBW], BF16, kind="ExternalInput")
    xt8_d = nc.dram_tensor("xt8", [NSB, 128, MC2, 2, SBW], F8, kind="ExternalInput")
    wq_d = nc.dram_tensor("wq", [PAIRS, 128, MC2, 2, 128], F8, kind="ExternalInput")
    wk_d = nc.dram_tensor("wk", [PAIRS, 128, MC2, 2, 128], F8, kind="ExternalInput")
    wv_d = nc.dram_tensor("wv", [128, MC, 512], BF16, kind="ExternalInput")
    wo_d = nc.dram_tensor("wo", [128, PAIRS, DM], BF16, kind="ExternalInput")
    bq_d = nc.dram_tensor("bq", [PAIRS, 128, 1], F32, kind="ExternalInput")
    bk_d = nc.dram_tensor("bk", [PAIRS, 128, 1], F32, kind="ExternalInput")
    msk_d = nc.dram_tensor("mask", [128, 256], BF16, kind="ExternalInput")
    out_d = nc.dram_tensor("out", [S, DM], BF16, kind="ExternalOutput")

    with tile.TileContext(nc) as tc:
      with (
          tc.tile_pool(name="hold", bufs=1) as hold,
          tc.tile_pool(name="ph2", bufs=1) as ph2,
          tc.tile_pool(name="patn", bufs=1, space="PSUM") as patn,
          tc.tile_pool(name="ph1", bufs=1) as ph1,
          tc.tile_pool(name="pqk", bufs=1, space="PSUM") as pqk,
      ):
        v_t = [hold.tile([128, NHC, DH + 1], BF16, tag=f"v{st}", name=f"v{st}") for st in range(NST)]
        z_t = [hold.tile([128, S], BF16, tag=f"z{p}", name=f"z{p}") for p in range(PAIRS)]
        msk_t = hold.tile([128, 256], BF16, tag="mtri")
        bq_t = [hold.tile([128, 1], F32, tag=f"bq{p}", name=f"bq{p}") for p in range(PAIRS)]
        bk_t = [hold.tile([128, 1], F32, tag=f"bk{p}", name=f"bk{p}") for p in range(PAIRS)]
        ones_c = hold.tile([128, 1], F32, tag="ones")
        qts = {}

        nc.vector.memset(ones_c[:], 1.0)
        # small constants go through the gpsimd DMA queue (only wo shares it,
        # and wo isn't needed until the last-pair phase) so their ~0.7us
        # triggers don't delay the xt/wv bulk loads on the sync/scalar queues
        nc.gpsimd.dma_start(msk_t[:], msk_d.ap())
        for p in range(PAIRS):
            nc.gpsimd.dma_start(bq_t[p][:], bq_d.ap()[p])
            nc.gpsimd.dma_start(bk_t[p][:], bk_d.ap()[p])

        # The PE HAM clock gate keeps the array at 1.2 GHz until it has been
        # continuously busy for a full ~3.4us activity window. The first few
        # us are framework preamble + DMA waits anyway, so run a continuous
        # stream of junk matmuls on a zeroed tile to flip the gate before the
        # real matmuls start (each N=512 matmul is ~426ns at the cold clock).
        warm_t = hold.tile([128, 512], BF16, tag="warm")
        nc.vector.memset(warm_t[:], 0.0)
        for _ in range(N_WARM):
            wp = patn.tile([128, 1024], F32, tag="sp", bufs=2, name="warm")
            nc.tensor.matmul(wp[0:16, 0:512], warm_t[:, 0:16], warm_t[:],
                             start=True, stop=True)

        def attn_scores(p, sb, j):
            """Scores + mask + exp for step j; PV is issued separately so
            filler matmuls can sit between them in the PE's strict FIFO,
            covering the exp latency."""
            qt, kt = qts[p]
            qtb = qt[sb]
            ktb = kt[j // 4]
            # columns q < j*128 of this key tile are fully masked;
            # restrict S/exp/PV to the valid suffix.
            j_rel = j - 4 * sb
            off = max(j_rel, 0) * 128
            sp = patn.tile([128, 1024], F32, tag="sp", bufs=2, name="sp")
            ks = ((j % 4) * 128, (j % 4 + 1) * 128)
            diag = j_rel >= 0
            nc.tensor.matmul(
                sp[:, off:512],
                ktb[0:64, ks[0]:ks[1]],
                qtb[0:64, off:SBW],
                start=True, stop=True,
                tile_position=(0, 0),
                skip_group_check=True,
            )
            nc.tensor.matmul(
                sp[:, 512 + off:1024],
                ktb[64:128, ks[0]:ks[1]],
                qtb[64:128, off:SBW],
                start=True, stop=True,
                tile_position=(64, 0),
                skip_group_check=True,
            )
            pt = ph2.tile([128, 1024], BF16, tag="pt", bufs=6, name="pt")
            if off == 0:
                # contiguous suffix: flat 2D AP is slightly cheaper on ACT
                nc.scalar.activation(pt[:, 0:1024], sp[:, 0:1024], EXPF, scale=SCALE)
            else:
                sp3 = sp[:].rearrange("p (u q) -> p u q", u=2)
                pt3 = pt[:].rearrange("p (u q) -> p u q", u=2)
                nc.scalar.activation(
                    pt3[:, :, off:512], sp3[:, :, off:512], EXPF, scale=SCALE
                )
            if diag:
                # zero the masked upper triangle of the diagonal prob block
                # for both heads with one DVE bf16 multiply (off the PE)
                pt3m = pt[:].rearrange("p (u q) -> p u q", u=2)
                tri3 = msk_t[:].rearrange("p (u q) -> p u q", u=2)
                nc.vector.tensor_mul(
                    pt3m[:, :, off:off + 128], pt3m[:, :, off:off + 128], tri3[:]
                )
            return pt, off

        def attn_pv(p, sb, j, z0, z1, pt, off):
            nkt = 4 * (sb + 1)
            nc.tensor.matmul(
                z0[:, off:512],
                v_t[j][:, 2 * p, :],
                pt[:, off:512],
                start=(j == 0), stop=(j == nkt - 1),
            )
            nc.tensor.matmul(
                z1[:, off:512],
                v_t[j][:, 2 * p + 1, :],
                pt[:, 512 + off:1024],
                start=(j == 0), stop=(j == nkt - 1),
            )

        def attn_j(p, sb, j, z0, z1, fill=None):
            pt, off = attn_scores(p, sb, j)
            if fill is not None:
                fill()
            attn_pv(p, sb, j, z0, z1, pt, off)

        norm_work = []

        def norm_finish(p, sb, s0, s1, c0=0, c1=512, direct=False):
            """The reciprocal/broadcast/multiply part of the normalize,
            returned as a list of small closures. Deferred finishes are
            drained ONE PIECE PER ATTENTION STEP (norm_pop): emitted as one
            block, their DVE ops sit in the DVE FIFO ahead of the
            diagonal-step mask multiplies and stall those steps' PVs ~2us
            (superblock 0 -- right after a pair boundary -- is ALL diagonal
            steps). The broadcasts stay on GPSIMD, which must run ONLY
            partition_broadcast: mixing in other gpsimd tensor ops makes the
            Q7 cores swap microcode libraries at ~6us per switch."""
            w = c1 - c0
            qs = (sb * SBW + c0, sb * SBW + c1)
            d0row = ph2.tile([1, 512], F32, tag="d0row", bufs=2, name="d0row")
            d1row = ph2.tile([1, 512], F32, tag="d1row", bufs=2, name="d1row")
            r0 = ph2.tile([64, 512], F32, tag="r0", bufs=2, name="r0")
            r1 = ph2.tile([64, 512], F32, tag="r1", bufs=2, name="r1")
            cc0, cc1 = (c0, c1) if direct else (0, w)

            def piece1():
                nc.vector.tensor_copy(d0row[:, 0:w], s0[DH:DH + 1, cc0:cc1])
                nc.vector.tensor_copy(d1row[:, 0:w], s1[DH:DH + 1, cc0:cc1])

            def piece2():
                nc.vector.reciprocal_approx_fast(d0row[:, 0:w], d0row[:, 0:w])
                nc.vector.reciprocal_approx_fast(d1row[:, 0:w], d1row[:, 0:w])
                nc.gpsimd.partition_broadcast(r0[:, 0:w], d0row[:, 0:w], channels=64)
                nc.gpsimd.partition_broadcast(r1[:, 0:w], d1row[:, 0:w], channels=64)

            def piece3():
                nc.vector.tensor_mul(
                    z_t[p][0:64, qs[0]:qs[1]], s0[0:64, cc0:cc1], r0[:, 0:w])

            def piece4():
                t1 = ph2.tile([64, 512], BF16, tag="t1", bufs=2, name="t1")
                nc.vector.tensor_mul(t1[:, 0:w], s1[0:64, cc0:cc1], r1[:, 0:w])
                nc.sync.dma_start(z_t[p][64:128, qs[0]:qs[1]], t1[:, 0:w])

            return [piece1, piece2, piece3, piece4]

        def attn_norm(p, sb, z0, z1, q0=0, q1=512, direct=False, defer=False):
            # normalize by the softmax denominator (row DH of z psum).
            # First copy z psum to SBUF so the bank frees immediately (the
            # next superblock's PV only waits for this copy, not the whole
            # reciprocal/broadcast/multiply chain). Optional [q0,q1) restricts
            # to a column chunk so the tail can pipeline norm with oproj.
            if direct:
                # tail chunks: nothing runs after, so bank release doesn't
                # matter -- read the z psum directly (shortest chain)
                for piece in norm_finish(p, sb, z0, z1, q0, q1, direct=True):
                    piece()
                return
            # copy the whole z psum (a [65,w] partition-parallel copy
            # costs ~the same as a [1,w] one) so each bank frees after
            # one DVE op instead of holding through the whole chain
            zc0 = ph2.tile([DH + 1, 512], F32, tag="zc0", bufs=2, name="zc0")
            zc1 = ph2.tile([DH + 1, 512], F32, tag="zc1", bufs=2, name="zc1")
            nc.vector.tensor_copy(zc0[:, 0:512], z0[:, 0:512])
            nc.vector.tensor_copy(zc1[:, 0:512], z1[:, 0:512])
            pieces = norm_finish(p, sb, zc0, zc1)
            if defer:
                norm_work.extend(pieces)
            else:
                for piece in pieces:
                    piece()

        def norm_pop(n=1):
            for _ in range(n):
                if norm_work:
                    norm_work.pop(0)()

        # xt in per-512-column-block tiles: attention(0, sb) needs only
        # Q/K columns <= (sb+1)*512 (causal), so the whole front of the
        # kernel pipelines by column block. Each block arrives as two big
        # DMAs (one per trigger queue); fp8 copies feed the QK DoubleRow
        # projections. fp8 column block 3 lives in the long-lived pool:
        # pair-3's sb-3 projection is deferred into the last-pair phase.
        xt_t = [[ph1.tile([128, MC // 2, SBW], BF16, tag=f"xt{h}_{cb}",
                          name=f"xt{h}_{cb}")
                 for h in range(2)] for cb in range(NSB)]
        xt8_t = [(hold if cb == 3 else ph1).tile(
                     [128, MC2, 2, SBW], F8, tag=f"x8{cb}", name=f"x8{cb}")
                 for cb in range(NSB)]
        wv_t = [ph1.tile([128, MC // 2, 512], BF16, tag=f"wv{h}", name=f"wv{h}")
                for h in range(2)]
        wo_t = ph1.tile([128, PAIRS, DM], BF16, tag="wo", name="wo")

        def qk_gen(p, sb_outer=False, defer_sb3=False):
            """QK projection for pair p (fp8 DoubleRow, pair-stacked
            partitions), yielded one matmul at a time for interleaving.
            Weight DMAs issue eagerly at creation; the matmuls come from the
            returned generator. With sb_outer=True the superblock loop is
            outermost so early superblocks finish as soon as their xt column
            block lands. defer_sb3 skips superblock 3 (emitted later, see
            last pair)."""
            qt = [hold.tile([128, SBW], mybir.dt.bfloat16, tag=f"qt{i}", bufs=2,
                            name=f"qt{i}")
                  for i in range(NSB)]
            kt = [hold.tile([128, SBW], mybir.dt.bfloat16, tag=f"kt{i}", bufs=2,
                            name=f"kt{i}")
                  for i in range(NSB)]
            qts[p] = (qt, kt)
            wqk = []
            for qi, (w_d, b_t, dst) in enumerate(
                    ((wq_d, bq_t, qt), (wk_d, bk_t, kt))):
                w = ph1.tile([128, MC2, 2, 128], F8, tag="w", bufs=4, name="w")
                (nc.sync if qi == 0 else nc.scalar).dma_start(w[:], w_d.ap()[p])
                wqk.append((w, b_t, dst))
            order = (
                [(sb, wb) for sb in range(NSB) for wb in wqk]
                if sb_outer else
                [(sb, wb) for wb in wqk for sb in range(NSB)]
            )
            if defer_sb3:
                order = [o for o in order if o[0] != 3]

            def gen():
                for sb, (w, b_t, dst) in order:
                    ps = pqk.tile([128, 512], F32, tag="acc", bufs=2, name="acc")
                    for mp in range(MC2):
                        nc.tensor.matmul(
                            ps[:],
                            w[:, mp, :, :],
                            xt8_t[sb][:, mp, :, :],
                            start=(mp == 0),
                            stop=(mp == MC2 - 1),
                            perf_mode=DROW,
                        )
                        yield
                    nc.vector.tensor_scalar_add(dst[sb][:], ps[:], b_t[p][:])
                    yield

            return gen()

        # column-block pipelined prelude: per block, land xt columns (two
        # half-block DMAs on separate trigger queues + one fp8 DMA), then
        # V-projection for its 4 seq tiles, pair 0's QK for it, and pair 0's
        # attention superblock cb. Interleaving pair-0 attention into the
        # prelude hides the next block's DMA latency (per-queue DMA delivery
        # is only ~0.25 MB/us) behind the ACT-paced softmax steps.
        # wv halves land first (the first V matmul needs them), wo much later
        nc.sync.dma_start(wv_t[0][:], wv_d.ap()[:, 0:MC // 2, :])
        nc.scalar.dma_start(wv_t[1][:], wv_d.ap()[:, MC // 2:MC, :])
        nc.gpsimd.dma_start(wo_t[:], wo_d.ap())
        g0 = qk_gen(0, sb_outer=True)
        state1 = {"done": False, "emitted": 0, "step": 0, "g": None, "cb": 0}

        def fill1(state=state1):
            # pair-1 projection filler inside pair-0's attention: capped by
            # the xt8 blocks that have landed (superblock-outer order, 10
            # items per block), front-loaded 2/step with 8 held back and 4
            # of those released over the last steps
            state["step"] += 1
            step = state["step"]
            want = min(2 * step, 10 * (state["cb"] + 1), 32) + max(0, step - 36)
            while state["emitted"] < want and not state["done"]:
                try:
                    next(state["g"])
                    state["emitted"] += 1
                except StopIteration:
                    state["done"] = True

        def issue_cb_dmas(cb):
            nc.sync.dma_start(xt_t[cb][0][:], xt_d.ap()[cb][:, 0:MC // 2, :])
            nc.scalar.dma_start(xt_t[cb][1][:], xt_d.ap()[cb][:, MC // 2:MC, :])
            (nc.scalar if cb % 2 == 0 else nc.sync).dma_start(
                xt8_t[cb][:], xt8_d.ap()[cb])

        issue_cb_dmas(0)
        for cb in range(NSB):
            if cb + 1 < NSB:
                # keep the next block's transfers in flight behind this
                # block's compute + attention
                issue_cb_dmas(cb + 1)
            for st in range(4 * cb, 4 * cb + 4):
                ps = pqk.tile([128, 512], F32, tag="acc", bufs=2, name="acc")
                for m in range(MC):
                    nc.tensor.matmul(
                        ps[:],
                        xt_t[cb][m // 4][:, m % 4,
                                         (st % 4) * 128:(st % 4 + 1) * 128],
                        wv_t[m // 4][:, m % 4, :],
                        start=(m == 0),
                        stop=(m == MC - 1),
                    )
                nc.vector.tensor_copy(
                    v_t[st][:, :, 0:DH],
                    ps[:].rearrange("p (h d) -> p h d", h=NHC),
                )
                nc.vector.tensor_copy(
                    v_t[st][:, :, DH],
                    ones_c[:].to_broadcast((128, NHC)),
                )
            for _ in range(10):  # one QK column-block (2 proj x (4 mm + copy))
                try:
                    next(g0)
                except StopIteration:
                    break
            if cb == 0:
                # pair 1's projection generator: weight DMAs issue now (they
                # queue behind block 0's bulk loads)
                state1["g"] = qk_gen(1, sb_outer=True)
            state1["cb"] = cb
            # pair 0's attention for superblock cb
            nkt = 4 * (cb + 1)
            z0 = patn.tile([DH + 1, 512], F32, tag="z0", bufs=1, name="z0")
            z1 = patn.tile([DH + 1, 512], F32, tag="z1", bufs=1, name="z1")
            for j in range(nkt):
                attn_j(0, cb, j, z0, z1, fill=fill1)
                norm_pop()
            attn_norm(0, cb, z0, z1, defer=True)
        for _ in g0:
            pass
        while not state1["done"]:
            try:
                next(state1["g"])
            except StopIteration:
                state1["done"] = True

        # attention for pairs 1-2, with pair p+1's projection matmuls
        # interleaved into the ACT-paced attention stream
        for p in (1, 2):
            g = qk_gen(p + 1, defer_sb3=(p == 2))
            total = 30 if p == 2 else 40
            state = {"done": False, "emitted": 0, "step": 0}

            def fill(state=state, g=g, total=total):
                # front-load 2/step, but hold 8 items back, releasing only 4
                # over the last steps: the remaining ~4 emit in the post-loop
                # drain, covering the pair-boundary normalize window
                state["step"] += 1
                step = state["step"]
                want = min(2 * step, total - 8) + max(0, step - 36)
                while state["emitted"] < want and not state["done"]:
                    try:
                        next(g)
                        state["emitted"] += 1
                    except StopIteration:
                        state["done"] = True

            for sb in range(NSB):
                nkt = 4 * (sb + 1)
                z0 = patn.tile([DH + 1, 512], F32, tag="z0", bufs=1, name="z0")
                z1 = patn.tile([DH + 1, 512], F32, tag="z1", bufs=1, name="z1")
                for j in range(nkt):
                    attn_j(p, sb, j, z0, z1, fill=fill)
                    norm_pop()
                attn_norm(p, sb, z0, z1, defer=True)
            while not state["done"]:
                try:
                    next(g)
                except StopIteration:
                    state["done"] = True

        # ---------------- last pair + output projection ----------------
        # pair-3's sb-3 Q/K projection was deferred out of pair 2's phase;
        # emit it here as filler for pair-3's otherwise-empty early steps.
        wq3 = ph1.tile([128, MC2, 2, 128], F8, tag="wq3", name="wq3")
        wk3 = ph1.tile([128, MC2, 2, 128], F8, tag="wk3", name="wk3")
        nc.scalar.dma_start(wq3[:], wq_d.ap()[3])
        nc.sync.dma_start(wk3[:], wk_d.ap()[3])

        def deferred_proj():
            for wts, b_t, dst in ((wq3, bq_t, qts[3][0]), (wk3, bk_t, qts[3][1])):
                ps = pqk.tile([128, 512], F32, tag="acc", bufs=2, name="dps")
                for mp in range(MC2):
                    nc.tensor.matmul(
                        ps[:],
                        wts[:, mp, :, :],
                        xt8_t[3][:, mp, :, :],
                        start=(mp == 0),
                        stop=(mp == MC2 - 1),
                        perf_mode=DROW,
                    )
                    yield
                nc.vector.tensor_scalar_add(dst[3][:], ps[:], b_t[3][:])
                yield

        gdef = deferred_proj()
        gstate = {"done": False}

        def oproj(q, mb, cp=None):
            # shares the projection psum tag, so units never contend with
            # pair-3's score psum
            ps = pqk.tile([128, 512], F32, tag="acc", bufs=2, name="ops")
            for p in range(PAIRS):
                nc.tensor.matmul(
                    ps[:],
                    z_t[p][:, q * 128:(q + 1) * 128],
                    wo_t[:, p, mb * 512:(mb + 1) * 512],
                    start=(p == 0),
                    stop=(p == PAIRS - 1),
                )
            ost = ph1.tile([128, 512], mybir.dt.bfloat16, tag="ost", bufs=4,
                           name="ost")
            (cp or nc.vector.tensor_copy)(ost[:], ps[:])
            nc.sync.dma_start(
                out_d.ap()[q * 128:(q + 1) * 128, mb * 512:(mb + 1) * 512],
                ost[:],
            )

        # pair 3's attention, with output-projection tiles for already-
        # complete superblocks interleaved in (sb lags by one).
        otodo = [(q, mb) for q in range(NST) for mb in range(2)]
        odone = 0
        for sb in range(NSB):
            nkt = 4 * (sb + 1)
            z0 = patn.tile([DH + 1, 512], F32, tag="z0", bufs=1, name="z0")
            z1 = patn.tile([DH + 1, 512], F32, tag="z1", bufs=1, name="z1")

            stepc = {"j": 0}

            def fill(sb=sb, nkt=nkt):
                # first finish the deferred sb-3 projection (2 matmuls per
                # step, exclusive so it doesn't contend with units for the
                # acc psum); then z for superblocks < sb is final for all
                # pairs -- spread this superblock's unit quota evenly over
                # its steps (instead of draining immediately and leaving
                # the later steps with no filler), holding 2 back so the
                # final norm chain has PE work to hide behind
                nonlocal odone
                stepc["j"] += 1
                pumped = 0
                for _ in range(2):
                    if not gstate["done"]:
                        try:
                            next(gdef)
                            pumped += 1
                        except StopIteration:
                            gstate["done"] = True
                if pumped == 0 and stepc["j"] >= 3:
                    # units wait until step 3 so the previous superblock's
                    # deferred norm pieces (needed by this superblock's
                    # q-tiles) have drained
                    ready = min(sb * 8, 22)
                    base = 8 * (sb - 1) if sb > 0 else 0
                    avail = ready - base
                    allowed = base + min(avail, (stepc["j"] * avail + nkt - 1) // nkt)
                    if odone < min(ready, allowed):
                        oproj(*otodo[odone])
                        odone += 1

            for j in range(nkt):
                attn_j(3, sb, j, z0, z1, fill=fill)
                # sb0 is all-diagonal: 1 piece/step keeps the mask muls near
                # the DVE FIFO head; later superblocks drain 2/step so each
                # norm is done before its q-tiles' output units are due
                norm_pop(1 if sb == 0 else 2)
            if sb < NSB - 1:
                attn_norm(3, sb, z0, z1, defer=True)
            else:
                norm_pop(4)
                # last superblock: chunk the normalize by 128 columns and
                # emit each q-tile's 2 output units right after its chunk,
                # so the norm chain pipelines with the final projections
                for c in range(4):
                    attn_norm(3, sb, z0, z1, q0=c * 128, q1=(c + 1) * 128, direct=True)
                    if c == 0:
                        # drain any interleave backlog (needs only sb<3 z,
                        # which is final) while chunk 0's norm chain runs;
                        # all endgame copies go to ScalarE (exp stream is
                        # done, and DVE must stay clear for the norm chain)
                        while odone < 24:
                            oproj(*otodo[odone], cp=nc.scalar.copy)
                            odone += 1
                    for i in range(2):
                        oproj(*otodo[odone],
                              cp=(nc.scalar.copy if i == 0 else None))
                        odone += 1

    nc.compile()
    return nc


def _get_nc():
    if "nc" not in _NC_CACHE:
        _NC_CACHE["nc"] = _build_nc()
    return _NC_CACHE["nc"]


def _causal_masks():
    # 0/1 keep-mask for the post-exp DVE multiply, one copy per head half
    k = np.arange(128)[:, None]
    q = np.arange(128)[None, :]
    tri = np.where(q >= k, 1.0, 0.0).astype(ml_dtypes.bfloat16)
    return np.concatenate([tri, tri], axis=1)


def kernel(resid_pre, W_Q, W_K, W_V, W_O, b_Q, b_K, b_V, b_O):
    global LAST_RESULTS
    resid_pre = np.asarray(resid_pre, dtype=np.float32)
    W_Q = np.asarray(W_Q, dtype=np.float32)
    W_K = np.asarray(W_K, dtype=np.float32)
    W_V = np.asarray(W_V, dtype=np.float32)
    W_O = np.asarray(W_O, dtype=np.float32)
    b_Q = np.asarray(b_Q, dtype=np.float32)
    b_K = np.asarray(b_K, dtype=np.float32)
    b_V = np.asarray(b_V, dtype=np.float32)
    b_O = np.asarray(b_O, dtype=np.float32)

    B = resid_pre.shape[0]
    masks = _causal_masks()
    BF = ml_dtypes.bfloat16
    F8NP = ml_dtypes.float8_e4m3

    def pack_xt(xt):  # [1024, 2048] -> [4, 128, 8, 512] bf16
        r = xt.reshape(MC, 128, NSB, SBW).transpose(2, 1, 0, 3)
        return np.ascontiguousarray(r).astype(BF)

    def pack_xt8(xt):  # [1024, 2048] -> [4, 128, 4, 2, 512] fp8
        r = xt.reshape(MC2, 2, 128, NSB, SBW).transpose(3, 2, 0, 1, 4)
        return np.ascontiguousarray(r).astype(F8NP)

    def pack_w8(w):  # [8, 1024, 64] -> [4, 128, 4, 2, 128] fp8, pre-scaled
        p8 = (w * WSCALE).reshape(PAIRS, 2, DM, DH).transpose(0, 2, 1, 3)
        p8 = p8.reshape(PAIRS, MC2, 2, 128, 128).transpose(0, 3, 1, 2, 4)
        return np.ascontiguousarray(p8).astype(F8NP)

    def pack_wv(w):  # [8, 1024, 64] -> [128, 8, 512] bf16
        r = w.transpose(1, 0, 2).reshape(DM, NHC * DH)
        r = r.reshape(MC, 128, NHC * DH).transpose(1, 0, 2)
        return np.ascontiguousarray(r).astype(BF)

    def pack_wo(w):  # [8, 64, 1024] -> [128, 4, 1024] bf16
        r = w.reshape(PAIRS, 128, DM).transpose(1, 0, 2)
        return np.ascontiguousarray(r).astype(BF)

    in_maps = []
    for c in range(8):
        b, g = divmod(c, 2)
        hs = slice(g * NHC, (g + 1) * NHC)
        xt = resid_pre[b].T
        in_maps.append({
            "xt": pack_xt(xt),
            "xt8": pack_xt8(xt),
            "wq": pack_w8(W_Q[hs]),
            "wk": pack_w8(W_K[hs]),
            "wv": pack_wv(W_V[hs]),
            "wo": pack_wo(W_O[hs]),
            "bq": np.ascontiguousarray(b_Q[hs].reshape(PAIRS, 128, 1)) * WSCALE,
            "bk": np.ascontiguousarray(b_K[hs].reshape(PAIRS, 128, 1)) * WSCALE,
            "mask": masks,
        })

    nc = _get_nc()
    res = bass_utils.run_bass_kernel_spmd(nc, in_maps, core_ids=list(range(8)))
    LAST_RESULTS = res

    # b_V contributes exactly sum_h W_O[h].T @ b_V[h] (softmax rows sum to 1)
    const = np.einsum("hdm,hd->m", W_O, b_V).astype(np.float32) + b_O
    out = np.empty((B, S, DM), dtype=np.float32)
    for b in range(B):
        out[b] = (res.results[2 * b]["out"].astype(np.float32)
                  + res.results[2 * b + 1]["out"].astype(np.float32) + const)
    return out
